# revision 32
# baseline (speedup 1.0000x reference)
"""Bass/Tile TRN2 kernel for additive-attention pooling.

Math per sample s:
    e = tanh(x[s] @ W + b)          # (T, 1)
    a = softmax(e, axis=0)          # over T
    y[s] = sum_t a[t] * x[s, t, :]  # (U,)

tanh is bounded in (-1, 1), so softmax needs no max-subtraction:
    p = exp(e);  y[s] = (sum_t p[t] x[s,t]) / (sum_t p[t])

Sharding: data-parallel over batch across 8 NeuronCores (32 samples each).

Per-core dataflow. x is streamed once, one sample per DMA: the 2048
timesteps are laid out q-packed (t = p*Q + q, Q=16) so partition p
receives 16 contiguous HBM rows = 32 KiB, one fully-contiguous 4 MiB
DMA per sample (near peak HBM BW, minimal descriptor overhead).

The kernel is a DVE/DMA co-bottleneck ("ridge"): the score dot-product
must stream all of x through an elementwise engine, and DVE is the only
engine that can multiply-by-tensor at line rate (GpSimd fights DVE for
the shared SBUF port; ACT has no tensor*tensor). To get DVE under the
DMA time per sample, score slices are computed two ways:
  - n_pair "pair" ops: one DVE tensor_mul over [128, 2*512] (products
    only, halves the per-op fixed overhead), then TWO reductions on ACT
    (accumulating copy) - ACT has independent ports and spare capacity.
  - remaining singles: fused DVE scalar_tensor_tensor with accum_out.
Weighted sum on PE, 4-way column-tiled (slice q -> col group q%4).
Finale batched per 8 samples: exp's accum_out drops row-sums into
column s%8 of a shared rs8 tile, so ONE denominator matmul, ONE
reciprocal, ONE scaled PSUM->SBUF copy and ONE 16 KiB output DMA
serve 8 samples (sel8_j selector matmuls route each sample's combined
row to PSUM partition s%8).

Two further refinements: (1) no PSUM memzero - the first matmul of
each col group uses a 32-wide stationary (its real column + a zeroed
tail of p_sc), writing the whole 32-row group so the bank is fully
defined before the combine copy; (2) the first/last two samples load
as two half-DMAs so their score ops overlap the second half's
transfer, trimming pipeline fill and drain.

Measured on 8 axon trn2 cores: ~372-442 us for earlier checkpoints
(large run-to-run device drift); this version measured ~42 us faster
than its predecessor back-to-back in the same device state, with DVE
and ACT balanced at ~9.6 us/sample each under a ~10.3-11 us/sample
DMA stream (4 MiB loads sustain ~380-407 GB/s/core; 2 MiB only ~341).
Baseline (per-superchunk, DVE-bound) was 442-449 us.
"""

from contextlib import ExitStack

import numpy as np

B, T, U = 256, 2048, 512
N_CORES = 8
B_LOC = B // N_CORES
P = 128

_BUILD_CACHE = {}


def _emit(ctx, tc, x, W, b, y, xbufs, n_pair, with_bias):
    from concourse import mybir

    nc = tc.nc
    f32 = mybir.dt.float32
    Alu = mybir.AluOpType
    Act = mybir.ActivationFunctionType

    b_loc, t_len, u = x.shape
    Q = t_len // P            # rows per partition per sample (16)
    n_single = Q - 2 * n_pair

    const = ctx.enter_context(tc.tile_pool(name="const", bufs=1))
    xp = ctx.enter_context(tc.tile_pool(name="xp", bufs=xbufs))
    scr = ctx.enter_context(tc.tile_pool(name="scr", bufs=3))
    scr_a = ctx.enter_context(tc.tile_pool(name="scr_a", bufs=2))
    ep = ctx.enter_context(tc.tile_pool(name="ep", bufs=4))
    sp = ctx.enter_context(tc.tile_pool(name="sp", bufs=4))
    op = ctx.enter_context(tc.tile_pool(name="op", bufs=4))
    ps_wb = ctx.enter_context(tc.tile_pool(name="ps_wb", bufs=1, space="PSUM"))
    ps_w = ctx.enter_context(tc.tile_pool(name="ps_w", bufs=4, space="PSUM"))
    ps_s = ctx.enter_context(tc.tile_pool(name="ps_s", bufs=1, space="PSUM"))
    ps_b = ctx.enter_context(tc.tile_pool(name="ps_b", bufs=2, space="PSUM"))

    # ---- constants ----
    # W as a [1, U] row, broadcast to all 128 partitions via a K=1 matmul,
    # then duplicated side by side so pair ops can read 2*U of weights.
    # (const loads go on the ACT HWDGE ring so the SP ring's first
    # instruction is sample 0's big load)
    w_row = const.tile([1, u], f32)
    nc.scalar.dma_start(w_row[:], W.rearrange("u o -> o u"))
    ones_row = const.tile([1, P], f32)
    nc.vector.memset(ones_row[:], 1.0)
    ones_col = const.tile([P, 1], f32)
    nc.vector.memset(ones_col[:], 1.0)
    wb_ps = ps_wb.tile([P, u], f32, tag="wb_ps")
    nc.tensor.matmul(wb_ps[:], ones_row[:], w_row[:], start=True, stop=True)
    Wb2 = const.tile([P, 2 * u], f32)
    nc.vector.tensor_copy(Wb2[:, 0:u], wb_ps[:])
    nc.vector.tensor_copy(Wb2[:, u:2 * u], wb_ps[:])
    # sel8 block j ([P, 8]): 1.0 at (partition 32i, col j) — sums the 4
    # col-group partial rows of sample s (s%8==j) onto PSUM partition j.
    # One tile holds all 8 blocks (a bufs=1 pool would alias 8 same-shape
    # tiles and deadlock the schedule).
    sel8_t = const.tile([P, 8 * 8], f32)
    nc.vector.memset(sel8_t[:], 0.0)
    for j in range(8):
        for i in range(4):
            nc.vector.memset(sel8_t[32 * i:32 * i + 1, 8 * j + j:8 * j + j + 1], 1.0)
    sel8 = [sel8_t[:, 8 * j:8 * (j + 1)] for j in range(8)]
    if with_bias:
        bt2 = const.tile([P, Q], f32)
        nc.scalar.dma_start(bt2[:], b.rearrange("(p q) o -> p (q o)",
                                                p=P, q=Q))

    xr = x.rearrange("s (p q) u -> s p (q u)", p=P, q=Q)

    for s in range(b_loc):
        g = s % 8
        # fat tile: whole sample, partition p holds Q contiguous HBM rows
        # -> one fully-contiguous 4 MiB DMA (4 MiB sustains ~407 GB/s vs
        # ~341 for 2 MiB transfers). The first and last two samples are
        # loaded as two half-DMAs instead: scores on the first half then
        # overlap the second half's transfer (subtile deps), shortening
        # the pipeline fill and the end-of-stream drain chain.
        xt = xp.tile([P, Q * u], f32)
        if s == 0 or s >= b_loc - 2:
            hw = (Q // 2) * u
            nc.sync.dma_start(xt[:, 0:hw], xr[s][:, 0:hw])
            nc.sync.dma_start(xt[:, hw:], xr[s][:, hw:])
        else:
            nc.sync.dma_start(xt[:], xr[s])

        wsum = ps_w.tile([P, u], f32, tag="wsum")

        if g == 0:
            rs8 = sp.tile([P, 8], f32, tag="rs8")
            bat = ps_b.tile([8, u], f32, tag="bat")

        e_sc = ep.tile([P, Q], f32, tag="e_sc")
        # pair slices: DVE multiplies 2 slices in one op (products only,
        # halves the per-op fixed overhead); ACT reduces each 512-block
        # with an accumulating copy (ACT reduce cost is dtype-independent
        # at ~1us, so fp32 products - no precision loss)
        for pq in range(n_pair):
            q0 = 2 * pq
            pr = scr.tile([P, 2 * u], f32)
            nc.vector.tensor_mul(pr[:], xt[:, q0 * u:(q0 + 2) * u], Wb2[:])
            for k in range(2):
                red = scr_a.tile([P, u], f32)
                nc.scalar.activation(red[:], pr[:, k * u:(k + 1) * u],
                                     Act.Copy,
                                     accum_out=e_sc[:, q0 + k:q0 + k + 1])
        # single slices: fused multiply+reduce on DVE
        for q in range(2 * n_pair, Q):
            sc = scr.tile([P, u], f32)
            nc.vector.scalar_tensor_tensor(
                out=sc[:],
                in0=xt[:, q * u:(q + 1) * u],
                scalar=1.0,
                in1=Wb2[:, 0:u],
                op0=Alu.mult,
                op1=Alu.mult,
                accum_out=e_sc[:, q:q + 1],
            )
        if with_bias:
            eb_sc = ep.tile([P, Q], f32, tag="eb_sc")
            nc.vector.tensor_add(eb_sc[:], e_sc[:], bt2[:])
        else:
            eb_sc = e_sc
        th_sc = ep.tile([P, Q], f32, tag="th_sc")
        nc.scalar.activation(th_sc[:], eb_sc[:], Act.Tanh)
        # p_sc gets a zeroed 32-col tail so the first matmul of each col
        # group can use a 32-wide stationary (real col + zeros): it then
        # writes its whole 32-row group, leaving the full PSUM bank
        # defined without an ACT memzero (ACT is the critical engine).
        p_sc = ep.tile([P, Q + 32], f32, tag="p_sc")
        nc.vector.memset(p_sc[:, Q:Q + 32], 0.0)
        nc.scalar.activation(p_sc[:, 0:Q], th_sc[:], Act.Exp,
                             accum_out=rs8[:, g:g + 1])

        # weighted sum: slice q -> col group q%4, partial at psum row 32j
        for q in range(Q):
            j = q % 4
            if q < 4:
                nc.tensor.matmul(
                    wsum[32 * j:32 * j + 32, :],
                    p_sc[:, q:q + 32],
                    xt[:, q * u:(q + 1) * u],
                    start=True, stop=False,
                    tile_position=(0, 32 * j),
                )
            else:
                nc.tensor.matmul(
                    wsum[32 * j:32 * j + 1, :],
                    p_sc[:, q:q + 1],
                    xt[:, q * u:(q + 1) * u],
                    start=False, stop=(q >= Q - 4),
                    tile_position=(0, 32 * j),
                )

        # combine the 4 partial rows and route to batch-PSUM partition g
        wsb = op.tile([P, u], f32, tag="wsb")
        nc.scalar.activation(wsb[:], wsum[:], Act.Copy)
        nc.tensor.matmul(bat[:], sel8[g], wsb[:],
                         start=(g == 0), stop=(g == 7))

        if g == 7:
            # denominators for the whole 8-group in one matmul
            s_ps = ps_s.tile([8, 1], f32)
            nc.tensor.matmul(s_ps[:], rs8[:], ones_col[:],
                             start=True, stop=True)
            inv8 = sp.tile([8, 1], f32, tag="inv8")
            nc.vector.reciprocal(inv8[:], s_ps[:])
            obat = op.tile([8, u], f32, tag="obat")
            nc.scalar.activation(obat[:], bat[:], Act.Copy, scale=inv8[:])
            nc.sync.dma_start(y[s - 7:s + 1, :], obat[:])


def build_nc(b_loc=B_LOC, t_len=T, u=U, xbufs=5, n_pair=5, with_bias=False):
    key = (b_loc, t_len, u, xbufs, n_pair, with_bias)
    if key in _BUILD_CACHE:
        return _BUILD_CACHE[key]
    import concourse.bacc as bacc
    import concourse.tile as tile
    from concourse import mybir

    nc = bacc.Bacc(
        "TRN2",
        target_bir_lowering=False,
        debug=False,
        num_devices=N_CORES,
    )
    x = nc.dram_tensor("x", [b_loc, t_len, u], mybir.dt.float32, kind="ExternalInput").ap()
    W = nc.dram_tensor("W", [u, 1], mybir.dt.float32, kind="ExternalInput").ap()
    b = nc.dram_tensor("b", [t_len, 1], mybir.dt.float32, kind="ExternalInput").ap()
    y = nc.dram_tensor("y", [b_loc, u], mybir.dt.float32, kind="ExternalOutput").ap()

    with tile.TileContext(nc) as tc:
        with ExitStack() as ctx:
            _emit(ctx, tc, x, W, b, y, xbufs, n_pair, with_bias)
    nc.compile()
    _BUILD_CACHE[key] = nc
    return nc


def kernel(x, W, b):
    x = np.ascontiguousarray(np.asarray(x, dtype=np.float32))
    W = np.ascontiguousarray(np.asarray(W, dtype=np.float32))
    b = np.ascontiguousarray(np.asarray(b, dtype=np.float32))
    assert x.shape == (B, T, U), x.shape

    from concourse.bass_utils import run_bass_kernel_spmd

    # b is all-zero in this problem's setup; the fast path skips the bias
    # add on the (bottleneck) DVE. A general build handles nonzero b.
    nc = build_nc(with_bias=bool(np.any(b)))
    in_maps = [
        {
            "x": np.ascontiguousarray(x[i * B_LOC:(i + 1) * B_LOC]),
            "W": W,
            "b": b,
        }
        for i in range(N_CORES)
    ]
    res = run_bass_kernel_spmd(nc, in_maps, core_ids=list(range(N_CORES)))
    return np.concatenate([r["y"] for r in res.results], axis=0)


# revision 35
# speedup vs baseline: 1.1546x; 1.1546x over previous
"""Bass/Tile TRN2 kernel for additive-attention pooling.

Math per sample s:
    e = tanh(x[s] @ W + b)          # (T, 1)
    a = softmax(e, axis=0)          # over T
    y[s] = sum_t a[t] * x[s, t, :]  # (U,)

tanh is bounded in (-1, 1), so softmax needs no max-subtraction:
    p = exp(e);  y[s] = (sum_t p[t] x[s,t]) / (sum_t p[t])

Sharding: data-parallel over batch across 8 NeuronCores (32 samples each).

Per-core dataflow. x is streamed once, one sample per DMA: the 2048
timesteps are laid out q-packed (t = p*Q + q, Q=16) so partition p
receives 16 contiguous HBM rows = 32 KiB, one fully-contiguous 4 MiB
DMA per sample (near peak HBM BW, minimal descriptor overhead).

The kernel is a DVE/DMA co-bottleneck ("ridge"): the score dot-product
must stream all of x through an elementwise engine, and DVE is the only
engine that can multiply-by-tensor at line rate (GpSimd fights DVE for
the shared SBUF port; ACT has no tensor*tensor). To get DVE under the
DMA time per sample, score slices are computed two ways:
  - n_pair "pair" ops: one DVE tensor_mul over [128, 2*512] (products
    only, halves the per-op fixed overhead), then TWO reductions on ACT
    (accumulating copy) - ACT has independent ports and spare capacity.
  - remaining singles: fused DVE scalar_tensor_tensor with accum_out.
Weighted sum on PE, 4-way column-tiled (slice q -> col group q%4).
Finale batched per 8 samples: exp's accum_out drops row-sums into
column s%8 of a shared rs8 tile, so ONE denominator matmul, ONE
reciprocal, ONE scaled PSUM->SBUF copy and ONE 16 KiB output DMA
serve 8 samples (sel8_j selector matmuls route each sample's combined
row to PSUM partition s%8).

Two further refinements: (1) no PSUM memzero - the first matmul of
each col group uses a 32-wide stationary (its real column + a zeroed
tail of p_sc), writing the whole 32-row group so the bank is fully
defined before the combine copy; (2) the first/last two samples load
as two half-DMAs so their score ops overlap the second half's
transfer, trimming pipeline fill and drain.

Measured on 8 axon trn2 cores: ~372-442 us for earlier checkpoints
(large run-to-run device drift); this version measured ~42 us faster
than its predecessor back-to-back in the same device state, with DVE
and ACT balanced at ~9.6 us/sample each under a ~10.3-11 us/sample
DMA stream (4 MiB loads sustain ~380-407 GB/s/core; 2 MiB only ~341).
Baseline (per-superchunk, DVE-bound) was 442-449 us.
"""

from contextlib import ExitStack

import numpy as np

B, T, U = 256, 2048, 512
N_CORES = 8
B_LOC = B // N_CORES
P = 128

_BUILD_CACHE = {}


def _emit(ctx, tc, x, W, b, y, xbufs, n_pair, with_bias):
    from concourse import mybir

    nc = tc.nc
    f32 = mybir.dt.float32
    Alu = mybir.AluOpType
    Act = mybir.ActivationFunctionType

    b_loc, t_len, u = x.shape
    Q = t_len // P            # rows per partition per sample (16)
    n_single = Q - 2 * n_pair

    const = ctx.enter_context(tc.tile_pool(name="const", bufs=1))
    xp = ctx.enter_context(tc.tile_pool(name="xp", bufs=xbufs))
    # 4 bufs: a pair-product tile is freed only after ACT's two reduces
    # consume it; with fewer bufs DVE's later score ops stall on ACT's
    # reduce progress mid-sample.
    scr = ctx.enter_context(tc.tile_pool(name="scr", bufs=4))
    scr_a = ctx.enter_context(tc.tile_pool(name="scr_a", bufs=2))
    ep = ctx.enter_context(tc.tile_pool(name="ep", bufs=4))
    sp = ctx.enter_context(tc.tile_pool(name="sp", bufs=4))
    op = ctx.enter_context(tc.tile_pool(name="op", bufs=2))
    ps_wb = ctx.enter_context(tc.tile_pool(name="ps_wb", bufs=1, space="PSUM"))
    ps_w = ctx.enter_context(tc.tile_pool(name="ps_w", bufs=4, space="PSUM"))
    ps_s = ctx.enter_context(tc.tile_pool(name="ps_s", bufs=1, space="PSUM"))
    ps_b = ctx.enter_context(tc.tile_pool(name="ps_b", bufs=2, space="PSUM"))

    # ---- constants ----
    # W as a [1, U] row, broadcast to all 128 partitions via a K=1 matmul,
    # then duplicated side by side so pair ops can read 2*U of weights.
    # (const loads go on the ACT HWDGE ring so the SP ring's first
    # instruction is sample 0's big load)
    w_row = const.tile([1, u], f32)
    nc.scalar.dma_start(w_row[:], W.rearrange("u o -> o u"))
    ones_row = const.tile([1, P], f32)
    nc.vector.memset(ones_row[:], 1.0)
    ones_col = const.tile([P, 1], f32)
    nc.vector.memset(ones_col[:], 1.0)
    wb_ps = ps_wb.tile([P, u], f32, tag="wb_ps")
    nc.tensor.matmul(wb_ps[:], ones_row[:], w_row[:], start=True, stop=True)
    Wb2 = const.tile([P, 2 * u], f32)
    nc.vector.tensor_copy(Wb2[:, 0:u], wb_ps[:])
    nc.vector.tensor_copy(Wb2[:, u:2 * u], wb_ps[:])
    # sel8 block j ([P, 8]): 1.0 at (partition 32i, col j) — sums the 4
    # col-group partial rows of sample s (s%8==j) onto PSUM partition j.
    # One tile holds all 8 blocks (a bufs=1 pool would alias 8 same-shape
    # tiles and deadlock the schedule).
    sel8_t = const.tile([P, 8 * 8], f32)
    nc.vector.memset(sel8_t[:], 0.0)
    for j in range(8):
        for i in range(4):
            nc.vector.memset(sel8_t[32 * i:32 * i + 1, 8 * j + j:8 * j + j + 1], 1.0)
    sel8 = [sel8_t[:, 8 * j:8 * (j + 1)] for j in range(8)]
    if with_bias:
        bt2 = const.tile([P, Q], f32)
        nc.scalar.dma_start(bt2[:], b.rearrange("(p q) o -> p (q o)",
                                                p=P, q=Q))

    xr = x.rearrange("s (p q) u -> s p (q u)", p=P, q=Q)

    for s in range(b_loc):
        g = s % 8
        # fat tile: whole sample, partition p holds Q contiguous HBM rows
        # -> one fully-contiguous 4 MiB DMA (4 MiB sustains ~407 GB/s vs
        # ~341 for 2 MiB transfers). The first and last two samples are
        # loaded as two half-DMAs instead: scores on the first half then
        # overlap the second half's transfer (subtile deps), shortening
        # the pipeline fill and the end-of-stream drain chain.
        xt = xp.tile([P, Q * u], f32)
        if s == 0 or s >= b_loc - 2:
            hw = (Q // 2) * u
            nc.sync.dma_start(xt[:, 0:hw], xr[s][:, 0:hw])
            nc.sync.dma_start(xt[:, hw:], xr[s][:, hw:])
        else:
            nc.sync.dma_start(xt[:], xr[s])

        wsum = ps_w.tile([P, u], f32, tag="wsum")

        if g == 0:
            rs8 = sp.tile([P, 8], f32, tag="rs8")
            bat = ps_b.tile([8, u], f32, tag="bat")

        e_sc = ep.tile([P, Q], f32, tag="e_sc")
        # pair slices: DVE multiplies 2 slices in one op (products only,
        # halves the per-op fixed overhead); ACT reduces each 512-block
        # with an accumulating copy (ACT reduce cost is dtype-independent
        # at ~1us, so fp32 products - no precision loss)
        for pq in range(n_pair):
            q0 = 2 * pq
            pr = scr.tile([P, 2 * u], f32)
            nc.vector.tensor_mul(pr[:], xt[:, q0 * u:(q0 + 2) * u], Wb2[:])
            for k in range(2):
                red = scr_a.tile([P, u], f32)
                nc.scalar.activation(red[:], pr[:, k * u:(k + 1) * u],
                                     Act.Copy,
                                     accum_out=e_sc[:, q0 + k:q0 + k + 1])
        # single slices: fused multiply+reduce on DVE
        for q in range(2 * n_pair, Q):
            sc = scr.tile([P, u], f32)
            nc.vector.scalar_tensor_tensor(
                out=sc[:],
                in0=xt[:, q * u:(q + 1) * u],
                scalar=1.0,
                in1=Wb2[:, 0:u],
                op0=Alu.mult,
                op1=Alu.mult,
                accum_out=e_sc[:, q:q + 1],
            )
        if with_bias:
            eb_sc = ep.tile([P, Q], f32, tag="eb_sc")
            nc.vector.tensor_add(eb_sc[:], e_sc[:], bt2[:])
        else:
            eb_sc = e_sc
        th_sc = ep.tile([P, Q], f32, tag="th_sc")
        nc.scalar.activation(th_sc[:], eb_sc[:], Act.Tanh)
        # p_sc gets a zeroed 32-col tail so the first matmul of each col
        # group can use a 32-wide stationary (real col + zeros): it then
        # writes its whole 32-row group, leaving the full PSUM bank
        # defined without an ACT memzero (ACT is the critical engine).
        p_sc = ep.tile([P, Q + 32], f32, tag="p_sc")
        nc.vector.memset(p_sc[:, Q:Q + 32], 0.0)
        nc.scalar.activation(p_sc[:, 0:Q], th_sc[:], Act.Exp,
                             accum_out=rs8[:, g:g + 1])

        # weighted sum: slice q -> col group q%4, partial at psum row 32j
        for q in range(Q):
            j = q % 4
            if q < 4:
                nc.tensor.matmul(
                    wsum[32 * j:32 * j + 32, :],
                    p_sc[:, q:q + 32],
                    xt[:, q * u:(q + 1) * u],
                    start=True, stop=False,
                    tile_position=(0, 32 * j),
                )
            else:
                nc.tensor.matmul(
                    wsum[32 * j:32 * j + 1, :],
                    p_sc[:, q:q + 1],
                    xt[:, q * u:(q + 1) * u],
                    start=False, stop=(q >= Q - 4),
                    tile_position=(0, 32 * j),
                )

        # combine the 4 partial rows and route to batch-PSUM partition g
        wsb = op.tile([P, u], f32, tag="wsb")
        nc.scalar.activation(wsb[:], wsum[:], Act.Copy)
        nc.tensor.matmul(bat[:], sel8[g], wsb[:],
                         start=(g == 0), stop=(g == 7))

        if g == 7:
            # denominators for the whole 8-group in one matmul
            s_ps = ps_s.tile([8, 1], f32)
            nc.tensor.matmul(s_ps[:], rs8[:], ones_col[:],
                             start=True, stop=True)
            inv8 = sp.tile([8, 1], f32, tag="inv8")
            nc.vector.reciprocal(inv8[:], s_ps[:])
            obat = op.tile([8, u], f32, tag="obat")
            nc.scalar.activation(obat[:], bat[:], Act.Copy, scale=inv8[:])
            nc.sync.dma_start(y[s - 7:s + 1, :], obat[:])


def build_nc(b_loc=B_LOC, t_len=T, u=U, xbufs=5, n_pair=5, with_bias=False):
    key = (b_loc, t_len, u, xbufs, n_pair, with_bias)
    if key in _BUILD_CACHE:
        return _BUILD_CACHE[key]
    import concourse.bacc as bacc
    import concourse.tile as tile
    from concourse import mybir

    nc = bacc.Bacc(
        "TRN2",
        target_bir_lowering=False,
        debug=False,
        num_devices=N_CORES,
    )
    x = nc.dram_tensor("x", [b_loc, t_len, u], mybir.dt.float32, kind="ExternalInput").ap()
    W = nc.dram_tensor("W", [u, 1], mybir.dt.float32, kind="ExternalInput").ap()
    b = nc.dram_tensor("b", [t_len, 1], mybir.dt.float32, kind="ExternalInput").ap()
    y = nc.dram_tensor("y", [b_loc, u], mybir.dt.float32, kind="ExternalOutput").ap()

    with tile.TileContext(nc) as tc:
        with ExitStack() as ctx:
            _emit(ctx, tc, x, W, b, y, xbufs, n_pair, with_bias)
    nc.compile()
    _BUILD_CACHE[key] = nc
    return nc


def kernel(x, W, b):
    x = np.ascontiguousarray(np.asarray(x, dtype=np.float32))
    W = np.ascontiguousarray(np.asarray(W, dtype=np.float32))
    b = np.ascontiguousarray(np.asarray(b, dtype=np.float32))
    assert x.shape == (B, T, U), x.shape

    from concourse.bass_utils import run_bass_kernel_spmd

    # b is all-zero in this problem's setup; the fast path skips the bias
    # add on the (bottleneck) DVE. A general build handles nonzero b.
    nc = build_nc(with_bias=bool(np.any(b)))
    in_maps = [
        {
            "x": np.ascontiguousarray(x[i * B_LOC:(i + 1) * B_LOC]),
            "W": W,
            "b": b,
        }
        for i in range(N_CORES)
    ]
    res = run_bass_kernel_spmd(nc, in_maps, core_ids=list(range(N_CORES)))
    return np.concatenate([r["y"] for r in res.results], axis=0)
